# revision 42
# baseline (speedup 1.0000x reference)
# Causal self-attention on 8 TRN2 NeuronCores.
#
# Sharding (data + tensor parallel per the hint):
#   core c -> batch b = c // 4, head group g = c % 4 (4 heads of 64 dims = 256).
#   Wq/Wk/Wv split column-wise per head group; Wo row-wise. Each core emits a
#   partial [D, S] output in bf16; the host sums the 4 partials per batch
#   (the "all-reduce" of row-parallel sharding), transposes, and adds
#   bo' = bo + bv @ Wo.T (the V-bias commutes through softmax-normalize +
#   out-projection, so it is folded into the host-side bias).
#
# Device kernel (per core), all matmuls bf16 (PE streams 1 col/cycle for
# every dtype, so bf16 costs the same PE time as fp32 but halves DMA/SBUF
# and unlocks 2x DVE modes; tolerance is 2e-2, bf16 lands ~1e-3):
#   xT [D, S] resident in SBUF (bf16).
#   QT/KT [128 = 2 heads x 64, pair, S] = W x; bias added on DVE during the
#     PSUM->SBUF stage via per-partition tensor_scalar_add (no bias matmuls;
#     1/8 softmax scale folded into Wq/bq on the host).
#   V [S, 4 heads, 64+1] with a ones column (rowsum rides along in PV).
#   Attention is one global chunk pipeline across all (pair, q-block)
#   blocks: per chunk, a row-tiled pair of K=64 score matmuls (two heads
#   concurrently in the PE array), exp on ACT (PSUM->SBUF bf16), causal
#   mask multiply on GpSimd (diag chunks), then PV accumulation.
#   The PE stream is emitted with scores lookahead 1 (scores of chunk i+1
#   precede PV of chunk i) and a calibrated amount of "filler" matmuls
#   (projections for later blocks + out-projection of finished q-blocks)
#   between them, so the PE never idles waiting for ACT and the HAM clock
#   gate stays at 2.4 GHz. ACT's exp stream is the secondary resource
#   (~68us vs ~100us PE); fillers are deferred so the late, ACT-heavy
#   blocks still have PE work available.
#   Normalization: rowsums -> PE broadcast (K=1 matmul) -> DVE approx
#   reciprocal -> multiply (head 0 on DVE in place, head 1 via tmp + DMA to
#   partitions 64..127).
#   Out projection: 2 accumulating matmuls per [128,512] tile, staged
#   PSUM->SBUF bf16 alternating DVE/ACT, DMA'd out bf16 (no bias on device).

import os

import numpy as np

S = 2048
D = 1024
DL = 256  # local head dims (4 heads x 64)
NCORES = 8

_cache = {}
LAST_EXEC_TIME_NS = None
LAST_TRACE_PATH = None


DEBUG = os.environ.get("KERNEL_DEBUG", "0") == "1"


def _build_bass():
    from concourse import bacc
    import concourse.tile as tile
    import concourse.mybir as mybir
    from concourse.bass import ts, ds

    f32 = mybir.dt.float32
    bf16 = mybir.dt.bfloat16
    Exp = mybir.ActivationFunctionType.Exp

    nc = bacc.Bacc("TRN2", target_bir_lowering=False, debug=False)

    # DRAM layouts mirror the SBUF tiles (partition-major, contiguous per
    # partition) so each input DMA lowers to 128 large descriptors instead
    # of thousands of 512B row fragments
    xT_d = nc.dram_tensor("xT", [128, 4, 8, 512], bf16, kind="ExternalInput")
    wqT_d = nc.dram_tensor("wqT", [128, 8, DL], bf16, kind="ExternalInput")
    wkT_d = nc.dram_tensor("wkT", [128, 8, DL], bf16, kind="ExternalInput")
    wvT_d = nc.dram_tensor("wvT", [128, 8, DL], bf16, kind="ExternalInput")
    woT_d = nc.dram_tensor("woT", [128, 2, D], bf16, kind="ExternalInput")
    bq_d = nc.dram_tensor("bq", [128, 2], f32, kind="ExternalInput")
    bk_d = nc.dram_tensor("bk", [128, 2], f32, kind="ExternalInput")
    mask_d = nc.dram_tensor("mask", [128, 128], bf16, kind="ExternalInput")
    out_d = nc.dram_tensor("outT", [D, S], bf16, kind="ExternalOutput")
    warm_d = nc.dram_tensor("warm", [2, 512], f32, kind="ExternalOutput")
    if DEBUG:
        qT_o = nc.dram_tensor("qT_o", [128, 2, S], bf16, kind="ExternalOutput")
        kT_o = nc.dram_tensor("kT_o", [128, 2, S], bf16, kind="ExternalOutput")
        v4_o = nc.dram_tensor("v4_o", [128, 16, 4, 65], bf16, kind="ExternalOutput")
        oT_o = nc.dram_tensor("oT_o", [128, 2, S], bf16, kind="ExternalOutput")

    with tile.TileContext(nc) as tc:
        with (
            tc.tile_pool(name="persist", bufs=1) as persist,
            tc.tile_pool(name="ptp", bufs=4) as ptp,
            tc.tile_pool(name="oup", bufs=2) as oup,
            tc.tile_pool(name="rbp", bufs=2) as rbp,
            tc.tile_pool(name="stp", bufs=3) as stp,
            tc.tile_pool(name="tbp", bufs=2) as tbp,
            tc.tile_pool(name="wsp", bufs=1) as wsp,
            tc.tile_pool(name="sc2", bufs=2, space="PSUM") as sc2,
            tc.tile_pool(name="mm", bufs=2, space="PSUM") as mm,
            tc.tile_pool(name="po", bufs=2, space="PSUM") as po,
        ):
            # ---- persistent SBUF tensors ----
            # xT is tb-major [p, tb, o, f2] so each quarter's DMA is one
            # contiguous 8KB descriptor per partition on both sides
            xT = persist.tile([128, 4, 8, 512], bf16, name="xT_sb")
            wqT = persist.tile([128, 8, DL], bf16, name="wqT_sb")
            wkT = persist.tile([128, 8, DL], bf16, name="wkT_sb")
            wvT = persist.tile([128, 8, DL], bf16, name="wvT_sb")
            woT = persist.tile([128, 2, D], bf16, name="woT_sb")
            bq = persist.tile([128, 2], f32, name="bq_sb")
            bk = persist.tile([128, 2], f32, name="bk_sb")
            mask = persist.tile([128, 128], bf16, name="mask_sb")
            ones_bf = persist.tile([128, 512], bf16, name="ones_bf")
            qT = persist.tile([128, 2, S], bf16, name="qT_sb")
            kT = persist.tile([128, 2, S], bf16, name="kT_sb")
            v4 = persist.tile([128, 16, 4, 65], bf16, name="v4_sb")
            oT = persist.tile([128, 2, S], bf16, name="oT_sb")

            # ---- input DMAs ----
            # gpsimd starts earliest and is otherwise idle: constants there
            nc.gpsimd.memset(ones_bf[:], 1.0)
            nc.gpsimd.memset(v4[:, :, :, 64:65], 1.0)
            # sync ring, in first-use order: wq+x(qb0) gate the pre-phase,
            # wk/wv before the rest of x, wo (out-proj) last
            nc.sync.dma_start(wqT[:], wqT_d.ap())
            # first quarter split per-mc so the first projection matmuls
            # start as soon as (wq, mc0) land rather than after all of tb0
            for mc in range(8):
                nc.sync.dma_start(xT[:, 0, mc], xT_d.ap()[:, 0, mc])
            nc.sync.dma_start(wkT[:], wkT_d.ap())
            nc.sync.dma_start(wvT[:], wvT_d.ap())
            for tb in range(1, 4):
                nc.sync.dma_start(xT[:, tb], xT_d.ap()[:, tb])
            nc.sync.dma_start(woT[:], woT_d.ap())
            # scalar ring: only tiny constants (a big transfer here would
            # stall the ACT sequencer mid-dma_start and block the exps)
            nc.scalar.dma_start(bq[:], bq_d.ap())
            nc.scalar.dma_start(bk[:], bk_d.ap())
            nc.scalar.dma_start(mask[:], mask_d.ap())

            # ---- ACT table preload: dummy exp while DMAs stream ----
            wexp = wsp.tile([1, 512], f32, name="wexp")
            nc.scalar.activation(wexp[:], ones_bf[0:1, :], Exp)
            nc.sync.dma_start(warm_d.ap()[1:2, :], wexp[:])

            # ---- PE warmup: keep the array busy (and HAM warming) until
            # the first projection's inputs arrive (~6-7us) ----
            NWARM = 8  # ~3.4us of cold matmuls: un-throttles HAM right as
            # the first projection inputs (wq + x mc0) land
            psW = mm.tile([128, 512], f32, tag="mm", name="psW")
            for i in range(NWARM):
                nc.tensor.matmul(
                    psW,
                    lhsT=ones_bf[:, 0:128],
                    rhs=ones_bf[:],
                    start=(i == 0),
                    stop=(i == NWARM - 1),
                    skip_group_check=True,
                )
            wstg = wsp.tile([1, 512], f32, name="wstg")
            nc.vector.tensor_copy(wstg[:], psW[0:1, :])
            nc.sync.dma_start(warm_d.ap()[0:1, :], wstg[:])

            # ---- filler units: (est_pe_ns, emit_fn) ----
            def qk_proj_units(wsb, bvec, dst, t, qb):
                cell = {}

                def mk(mc):
                    def fn():
                        if mc == 0:
                            cell["ps"] = mm.tile(
                                [128, 512], f32, tag="mm", name="psqk"
                            )
                        nc.tensor.matmul(
                            cell["ps"],
                            lhsT=wsb[:, mc, ts(t, 128)],
                            rhs=xT[:, qb, mc, :],
                            start=(mc == 0),
                            stop=(mc == 7),
                            skip_group_check=True,
                        )
                        if mc == 7:
                            nc.vector.tensor_scalar_add(
                                dst[:, t, ts(qb, 512)],
                                cell["ps"],
                                bvec[:, t : t + 1],
                            )

                    return (270, fn)

                return [mk(mc) for mc in range(8)]

            def v_proj_units(st):
                cell = {}

                def mk(mc):
                    def fn():
                        if mc == 0:
                            cell["ps"] = mm.tile(
                                [128, 512], f32, tag="mm", name="psv"
                            )
                        nc.tensor.matmul(
                            cell["ps"][:, 0:256],
                            lhsT=xT[:, st // 4, mc, ts(st % 4, 128)],
                            rhs=wvT[:, mc, :],
                            start=(mc == 0),
                            stop=(mc == 7),
                            skip_group_check=True,
                        )
                        if mc == 7:
                            nc.vector.tensor_copy(
                                v4[:, st, :, 0:64],
                                cell["ps"][:, 0:256].rearrange(
                                    "p (h d) -> p h d", h=4
                                ),
                            )

                    return (160, fn)

                return [mk(mc) for mc in range(8)]

            op_count = [0]

            def outproj_units(sb):
                units = []
                for jt in range(8):

                    def fn(jt=jt):
                        ps = mm.tile([128, 512], f32, tag="mm", name="psop")
                        for dchunk in range(2):
                            nc.tensor.matmul(
                                ps,
                                lhsT=woT[:, dchunk, ts(jt, 128)],
                                rhs=oT[:, dchunk, ts(sb, 512)],
                                start=(dchunk == 0),
                                stop=(dchunk == 1),
                                skip_group_check=True,
                            )
                        stg = stp.tile([128, 512], bf16, tag="st", name="stg")
                        nc.vector.tensor_copy(stg[:], ps)
                        # alternate HWDGE rings so the final output DMAs
                        # drain two-wide instead of piling on one FIFO
                        dma_eng = nc.sync if jt % 2 == 0 else nc.scalar
                        dma_eng.dma_start(
                            out_d.ap()[ts(jt, 128), ts(sb, 512)], stg[:]
                        )

                    units.append((560, fn))
                return units

            filler = []  # list of (cost, fn), consumed front-first
            consumed = [0]

            # hold back a few units so the PE still has queued work during
            # the final block's normalization chain (released at the end)
            reserve = [5]

            def drain(budget_ns):
                spent = 0
                while len(filler) > reserve[0] and spent < budget_ns:
                    cost, fn = filler.pop(0)
                    fn()
                    consumed[0] += 1
                    spent += cost

            def drain_until(count):
                # force-consume prerequisite units: a block's scores may
                # never be emitted into the PE FIFO ahead of the filler
                # matmuls that produce its Q/K/V (in-order queue deadlock)
                while filler and consumed[0] < count:
                    cost, fn = filler.pop(0)
                    fn()
                    consumed[0] += 1

            # ---- attention chunk pipeline ----
            class Ch:
                __slots__ = (
                    "pair", "qb", "c", "w", "q0", "dc",
                    "first", "last", "ps2", "pt",
                )

            # block order (0,2,3,1): the ACT-heaviest qb3 blocks sit
            # mid-schedule where filler (qb1 projections + unlocked
            # out-proj) is plentiful; the final blocks are the small qb1
            # ones, so the PE never starves late and HAM stays at 8/8
            chunks = []
            for qb in (0, 2, 3, 1):
                for pair in range(2):
                    nch = 4 * qb + 4
                    for c in range(nch):
                        ch = Ch()
                        ch.pair, ch.qb, ch.c = pair, qb, c
                        dc = c - 4 * qb
                        ch.dc = dc
                        ch.q0 = 128 * dc if dc >= 0 else 0
                        ch.w = 512 - ch.q0
                        ch.first = c == 0
                        ch.last = c == nch - 1
                        chunks.append(ch)

            def emit_scores(ch):
                ps2 = sc2.tile([128, 2, 512], f32, tag="sc", name="ps2")
                for hh in (0, 1):
                    prow = slice(64 * hh, 64 * hh + 64)
                    nc.tensor.matmul(
                        ps2[:, hh, : ch.w],
                        lhsT=kT[prow, ch.pair, ts(ch.c, 128)],
                        rhs=qT[prow, ch.pair, ds(512 * ch.qb + ch.q0, ch.w)],
                        start=True,
                        stop=True,
                    )
                ch.ps2 = ps2

            def emit_exp(ch):
                pt = ptp.tile([128, 2, 512], bf16, tag="pt", name="pt")
                nc.scalar.activation(pt[:, :, : ch.w], ch.ps2[:, :, : ch.w], Exp)
                if ch.dc >= 0:
                    nc.gpsimd.tensor_mul(
                        pt[:, :, 0:128],
                        pt[:, :, 0:128],
                        mask[:, None, :].to_broadcast((128, 2, 128)),
                    )
                ch.pt = pt

            blk = {}

            def emit_pv(ch):
                if ch.first:
                    blk["psA"] = po.tile([128, 512], f32, tag="po", name="psA")
                    blk["psB"] = po.tile([128, 512], f32, tag="po", name="psB")
                for hh, psO in ((0, blk["psA"]), (1, blk["psB"])):
                    nc.tensor.matmul(
                        psO[0:65, ds(ch.q0, ch.w)],
                        lhsT=v4[:, ch.c, 2 * ch.pair + hh, :],
                        rhs=ch.pt[:, hh, : ch.w],
                        start=ch.first,
                        stop=ch.last,
                        skip_group_check=True,
                    )

            def emit_norm_copies():
                # PSUM->SBUF copies issued right after the block's last PV
                # so the po slots free up quickly; the PE-side broadcast
                # matmuls are deferred past the next filler batch so the PE
                # isn't parked in FIFO behind these DVE copies
                psA, psB = blk["psA"], blk["psB"]
                oA = oup.tile([128, 512], bf16, tag="ou", name="oA")
                oB = oup.tile([128, 512], bf16, tag="ou", name="oB")
                nc.vector.tensor_copy(oA[0:65, :], psA[0:65, :])
                nc.vector.tensor_copy(oB[0:65, :], psB[0:65, :])
                return oA, oB

            def emit_norm_rest(pair, qb, oA, oB, last=False):
                psR = po.tile([128, 512], f32, tag="po", name="psR")
                nc.tensor.matmul(
                    psR[0:64, :],
                    lhsT=ones_bf[64:65, 0:64],
                    rhs=oA[64:65, :],
                    start=True,
                    stop=True,
                    skip_group_check=True,
                )
                rbA = rbp.tile([128, 512], f32, tag="rb", name="rbA")
                nc.vector.reciprocal_approx_fast(rbA[0:64, :], psR[0:64, :])
                psR2 = po.tile([128, 512], f32, tag="po", name="psR2")
                nc.tensor.matmul(
                    psR2[0:64, :],
                    lhsT=ones_bf[64:65, 0:64],
                    rhs=oB[64:65, :],
                    start=True,
                    stop=True,
                    skip_group_check=True,
                )
                rbB = rbp.tile([128, 512], f32, tag="rb", name="rbB")
                nc.vector.reciprocal_approx_fast(rbB[0:64, :], psR2[0:64, :])
                # gpsimd mul is ~2x slower than DVE but off the critical
                # path mid-kernel; for the final block the chain gates the
                # last out-proj, so use DVE there
                mul_eng = nc.vector if last else nc.gpsimd
                mul_eng.tensor_mul(
                    oT[0:64, pair, ts(qb, 512)], oA[0:64, :], rbA[0:64, :]
                )
                tmpB = tbp.tile([128, 512], bf16, tag="tb", name="tmpB")
                mul_eng.tensor_mul(tmpB[0:64, :], oB[0:64, :], rbB[0:64, :])
                nc.sync.dma_start(oT[64:128, pair, ts(qb, 512)], tmpB[0:64, :])
                if pair == 1:
                    filler.extend(outproj_units(qb))

            # ---- pre-phase: minimal projections for (pair0, qb0, chunk0) ----
            for u in qk_proj_units(wqT, bq, qT, 0, 0):
                u[1]()
            for u in qk_proj_units(wkT, bk, kT, 0, 0):
                u[1]()
            for u in v_proj_units(0):
                u[1]()

            # ---- filler schedule (dependency-ordered for block order
            # 0,2,3,1). NOTE the asymmetry: Q is per-(pair, q-block), but
            # K is per-(pair, K-RANGE) — block (p, qb) reads kT columns
            # 0..(4qb+4)*128, i.e. K ranges 0..qb cumulatively — and V is
            # per-st chunk 0..4qb+3. Out-proj units are appended as their
            # q-block completes and consumed in the lulls that follow. ----
            for st in range(1, 4):  # V st1-3: per-chunk prereq of (0,0)
                filler += v_proj_units(st)
            filler += qk_proj_units(wqT, bq, qT, 1, 0)  # before (1,0)
            filler += qk_proj_units(wkT, bk, kT, 1, 0)
            # before (0,2): Q t0 qb2, K t0 ranges 1-2, V st4-11
            filler += qk_proj_units(wqT, bq, qT, 0, 2)
            filler += qk_proj_units(wkT, bk, kT, 0, 1)
            filler += qk_proj_units(wkT, bk, kT, 0, 2)
            for st in range(4, 12):
                filler += v_proj_units(st)
            # before (1,2): Q t1 qb2, K t1 ranges 1-2
            filler += qk_proj_units(wqT, bq, qT, 1, 2)
            filler += qk_proj_units(wkT, bk, kT, 1, 1)
            filler += qk_proj_units(wkT, bk, kT, 1, 2)
            # before (0,3): Q t0 qb3, K t0 range 3, V st12-15
            filler += qk_proj_units(wqT, bq, qT, 0, 3)
            filler += qk_proj_units(wkT, bk, kT, 0, 3)
            for st in range(12, 16):
                filler += v_proj_units(st)
            # before (1,3): Q t1 qb3, K t1 range 3
            filler += qk_proj_units(wqT, bq, qT, 1, 3)
            filler += qk_proj_units(wkT, bk, kT, 1, 3)
            # before (0,1)/(1,1): just Q (K ranges 0-1 already done)
            filler += qk_proj_units(wqT, bq, qT, 0, 1)
            filler += qk_proj_units(wqT, bq, qT, 1, 1)

            # units that must be consumed before each block's first scores
            # (cumulative position in the dependency-ordered filler list)
            prereq = {
                (1, 0): 40,
                (0, 2): 128,
                (1, 2): 152,
                (0, 3): 200,
                (1, 3): 216,
                (0, 1): 224,
                (1, 1): 232,
            }
            # V st1-3 sit at filler positions 0..23; (p0,qb0) chunk c's PV
            # needs V st c emitted first (PE FIFO would deadlock otherwise)
            pv_prereq = {(0, 0, 1): 8, (0, 0, 2): 16, (0, 0, 3): 24}

            emit_scores(chunks[0])
            emit_exp(chunks[0])
            nchunks_total = len(chunks)
            debt = [0.0]
            pending = []
            for i, ch in enumerate(chunks):
                if i + 1 < len(chunks):
                    nxt = chunks[i + 1]
                    if nxt.first and (nxt.pair, nxt.qb) in prereq:
                        drain_until(prereq[(nxt.pair, nxt.qb)])
                    emit_scores(nxt)
                    emit_exp(nxt)
                key = (ch.pair, ch.qb, ch.c)
                if key in pv_prereq:
                    drain_until(pv_prereq[key])
                # keep PE fed while ACT computes exp(ch): spread the
                # remaining filler evenly over the remaining chunks so the
                # late ACT-heavy blocks never starve the PE. Accumulate the
                # budget and release it in >=1.2us batches: consecutive
                # same-shape matmuls keep LDWEIGHTS prefetch working (a
                # lone filler MM after a PV pays its weight load exposed).
                remaining = sum(c for c, _ in filler)
                left = nchunks_total - i
                debt[0] += max(300 + 0.45 * ch.w, 1.05 * remaining / left)
                if debt[0] >= 1200 or left <= 2:
                    drain(debt[0])
                    debt[0] = 0.0
                while pending:
                    emit_norm_rest(*pending.pop(0))
                emit_pv(ch)
                if ch.last:
                    oA, oB = emit_norm_copies()
                    pending.append(
                        (ch.pair, ch.qb, oA, oB, i == nchunks_total - 1)
                    )

            # tail: a bit of reserved PE work covers the final norm chain,
            # then the rest of the out-proj (at least the last q-block's)
            reserve[0] = 0
            drain(2500)
            while pending:
                emit_norm_rest(*pending.pop(0))
            while filler:
                filler.pop(0)[1]()

            if DEBUG:
                nc.sync.dma_start(qT_o.ap(), qT[:])
                nc.sync.dma_start(kT_o.ap(), kT[:])
                nc.sync.dma_start(v4_o.ap(), v4[:])
                nc.sync.dma_start(oT_o.ap(), oT[:])

    nc.compile()
    return nc


def _get_bass():
    if "nc" not in _cache:
        _cache["nc"] = _build_bass()
    return _cache["nc"]


def _shard_inputs(x, Wq, bq, Wk, bk, Wv, bv, Wo, bo):
    import ml_dtypes

    bft = ml_dtypes.bfloat16
    x = np.asarray(x, dtype=np.float32)
    Wq = np.asarray(Wq, dtype=np.float32)
    Wk = np.asarray(Wk, dtype=np.float32)
    Wv = np.asarray(Wv, dtype=np.float32)
    Wo = np.asarray(Wo, dtype=np.float32)
    bq = np.asarray(bq, dtype=np.float32)
    bk = np.asarray(bk, dtype=np.float32)

    kk = np.arange(128)[:, None]
    qq = np.arange(128)[None, :]
    mask128 = (kk <= qq).astype(bft)

    def dev_layout(mat, npart_groups):
        # [G*128, F] -> [128, G, F] (partition-major, contiguous/partition)
        g, f = npart_groups, mat.shape[1]
        return np.ascontiguousarray(
            mat.reshape(g, 128, f).transpose(1, 0, 2)
        )

    def x_layout(xTb):
        # [1024, 2048] -> [128, 4 tb, 8 o, 512]
        t = xTb.reshape(8, 128, 4, 512).transpose(1, 2, 0, 3)
        return np.ascontiguousarray(t)

    xT = [x_layout(x[b].T.astype(bft)) for b in range(x.shape[0])]
    in_maps = []
    for c in range(NCORES):
        b, g = divmod(c, 4)
        sl = slice(DL * g, DL * (g + 1))
        in_maps.append(
            {
                "xT": xT[b],
                "wqT": dev_layout((Wq[sl].T * 0.125).astype(bft), 8),
                "wkT": dev_layout(Wk[sl].T.astype(bft), 8),
                "wvT": dev_layout(Wv[sl].T.astype(bft), 8),
                "woT": dev_layout(Wo[:, sl].T.astype(bft), 2),
                "bq": np.ascontiguousarray(
                    (bq[sl] * 0.125).reshape(2, 128).T
                ),
                "bk": np.ascontiguousarray(bk[sl].reshape(2, 128).T),
                "mask": mask128,
            }
        )
    return in_maps


def kernel(x, Wq, bq, Wk, bk, Wv, bv, Wo, bo):
    global LAST_EXEC_TIME_NS, LAST_TRACE_PATH
    from concourse.bass_utils import run_bass_kernel_spmd

    nc = _get_bass()
    in_maps = _shard_inputs(x, Wq, bq, Wk, bk, Wv, bv, Wo, bo)

    trace = os.environ.get("KERNEL_TRACE", "0") == "1"
    res = run_bass_kernel_spmd(
        nc, in_maps, core_ids=list(range(NCORES)), trace=trace
    )
    LAST_EXEC_TIME_NS = res.exec_time_ns
    if res.instructions_and_trace is not None:
        LAST_TRACE_PATH = res.instructions_and_trace[1]

    bo_full = (
        np.asarray(bo, np.float64)
        + np.asarray(bv, np.float64) @ np.asarray(Wo, np.float64).T
    ).astype(np.float32)

    B = 2
    out = np.empty((B, S, D), dtype=np.float32)
    for b in range(B):
        acc = res.results[4 * b]["outT"].astype(np.float32)
        for g in range(1, 4):
            acc = acc + res.results[4 * b + g]["outT"].astype(np.float32)
        out[b] = acc.T + bo_full[None, :]
    return out


# revision 45
# speedup vs baseline: 1.0251x; 1.0251x over previous
# Causal self-attention on 8 TRN2 NeuronCores.
#
# Sharding (data + tensor parallel per the hint):
#   core c -> batch b = c // 4, head group g = c % 4 (4 heads of 64 dims = 256).
#   Wq/Wk/Wv split column-wise per head group; Wo row-wise. Each core emits a
#   partial [D, S] output in bf16; the host sums the 4 partials per batch
#   (the "all-reduce" of row-parallel sharding), transposes, and adds
#   bo' = bo + bv @ Wo.T (the V-bias commutes through softmax-normalize +
#   out-projection, so it is folded into the host-side bias).
#
# Device kernel (per core), all matmuls bf16 (PE streams 1 col/cycle for
# every dtype, so bf16 costs the same PE time as fp32 but halves DMA/SBUF
# and unlocks 2x DVE modes; tolerance is 2e-2, bf16 lands ~1e-3):
#   xT [D, S] resident in SBUF (bf16).
#   QT/KT [128 = 2 heads x 64, pair, S] = W x; bias added on DVE during the
#     PSUM->SBUF stage via per-partition tensor_scalar_add (no bias matmuls;
#     1/8 softmax scale folded into Wq/bq on the host).
#   V [S, 4 heads, 64+1] with a ones column (rowsum rides along in PV).
#   Attention is one global chunk pipeline across all (pair, q-block)
#   blocks: per chunk, a row-tiled pair of K=64 score matmuls (two heads
#   concurrently in the PE array), exp on ACT (PSUM->SBUF bf16), causal
#   mask multiply on GpSimd (diag chunks), then PV accumulation.
#   The PE stream is emitted with scores lookahead 1 (scores of chunk i+1
#   precede PV of chunk i) and a calibrated amount of "filler" matmuls
#   (projections for later blocks + out-projection of finished q-blocks)
#   between them, so the PE never idles waiting for ACT and the HAM clock
#   gate stays at 2.4 GHz. ACT's exp stream is the secondary resource
#   (~68us vs ~100us PE); fillers are deferred so the late, ACT-heavy
#   blocks still have PE work available.
#   Normalization: rowsums -> PE broadcast (K=1 matmul) -> DVE approx
#   reciprocal -> multiply (head 0 on DVE in place, head 1 via tmp + DMA to
#   partitions 64..127).
#   Out projection: 2 accumulating matmuls per [128,512] tile, staged
#   PSUM->SBUF bf16 alternating DVE/ACT, DMA'd out bf16 (no bias on device).

import os

import numpy as np

S = 2048
D = 1024
DL = 256  # local head dims (4 heads x 64)
NCORES = 8

_cache = {}
LAST_EXEC_TIME_NS = None
LAST_TRACE_PATH = None


DEBUG = os.environ.get("KERNEL_DEBUG", "0") == "1"


def _build_bass():
    from concourse import bacc
    import concourse.tile as tile
    import concourse.mybir as mybir
    from concourse.bass import ts, ds

    f32 = mybir.dt.float32
    bf16 = mybir.dt.bfloat16
    Exp = mybir.ActivationFunctionType.Exp

    nc = bacc.Bacc("TRN2", target_bir_lowering=False, debug=False)

    # DRAM layouts mirror the SBUF tiles (partition-major, contiguous per
    # partition) so each input DMA lowers to 128 large descriptors instead
    # of thousands of 512B row fragments
    xT_d = nc.dram_tensor("xT", [128, 4, 8, 512], bf16, kind="ExternalInput")
    wqT_d = nc.dram_tensor("wqT", [128, 8, DL], bf16, kind="ExternalInput")
    wkT_d = nc.dram_tensor("wkT", [128, 8, DL], bf16, kind="ExternalInput")
    wvT_d = nc.dram_tensor("wvT", [128, 8, DL], bf16, kind="ExternalInput")
    woT_d = nc.dram_tensor("woT", [128, 2, D], bf16, kind="ExternalInput")
    bq_d = nc.dram_tensor("bq", [128, 2], f32, kind="ExternalInput")
    bk_d = nc.dram_tensor("bk", [128, 2], f32, kind="ExternalInput")
    mask_d = nc.dram_tensor("mask", [128, 128], bf16, kind="ExternalInput")
    out_d = nc.dram_tensor("outT", [D, S], bf16, kind="ExternalOutput")
    warm_d = nc.dram_tensor("warm", [2, 512], f32, kind="ExternalOutput")
    if DEBUG:
        qT_o = nc.dram_tensor("qT_o", [128, 2, S], bf16, kind="ExternalOutput")
        kT_o = nc.dram_tensor("kT_o", [128, 2, S], bf16, kind="ExternalOutput")
        v4_o = nc.dram_tensor("v4_o", [128, 16, 4, 65], bf16, kind="ExternalOutput")
        oT_o = nc.dram_tensor("oT_o", [128, 2, S], bf16, kind="ExternalOutput")

    with tile.TileContext(nc) as tc:
        with (
            tc.tile_pool(name="persist", bufs=1) as persist,
            tc.tile_pool(name="ptp", bufs=4) as ptp,
            tc.tile_pool(name="oup", bufs=2) as oup,
            tc.tile_pool(name="rbp", bufs=2) as rbp,
            tc.tile_pool(name="stp", bufs=3) as stp,
            tc.tile_pool(name="tbp", bufs=2) as tbp,
            tc.tile_pool(name="wsp", bufs=1) as wsp,
            tc.tile_pool(name="sc2", bufs=2, space="PSUM") as sc2,
            tc.tile_pool(name="mm", bufs=2, space="PSUM") as mm,
            tc.tile_pool(name="po", bufs=2, space="PSUM") as po,
        ):
            # ---- persistent SBUF tensors ----
            # xT is tb-major [p, tb, o, f2] so each quarter's DMA is one
            # contiguous 8KB descriptor per partition on both sides
            xT = persist.tile([128, 4, 8, 512], bf16, name="xT_sb")
            wqT = persist.tile([128, 8, DL], bf16, name="wqT_sb")
            wkT = persist.tile([128, 8, DL], bf16, name="wkT_sb")
            wvT = persist.tile([128, 8, DL], bf16, name="wvT_sb")
            woT = persist.tile([128, 2, D], bf16, name="woT_sb")
            bq = persist.tile([128, 2], f32, name="bq_sb")
            bk = persist.tile([128, 2], f32, name="bk_sb")
            mask = persist.tile([128, 128], bf16, name="mask_sb")
            ones_bf = persist.tile([128, 512], bf16, name="ones_bf")
            qT = persist.tile([128, 2, S], bf16, name="qT_sb")
            kT = persist.tile([128, 2, S], bf16, name="kT_sb")
            v4 = persist.tile([128, 16, 4, 65], bf16, name="v4_sb")
            oT = persist.tile([128, 2, S], bf16, name="oT_sb")

            # ---- input DMAs ----
            # gpsimd starts earliest and is otherwise idle: constants there
            nc.gpsimd.memset(ones_bf[:], 1.0)
            nc.gpsimd.memset(v4[:, :, :, 64:65], 1.0)
            # sync ring, in first-use order: wq+x(qb0) gate the pre-phase,
            # wk/wv before the rest of x, wo (out-proj) last
            nc.sync.dma_start(wqT[:], wqT_d.ap())
            nc.sync.dma_start(xT[:, 0], xT_d.ap()[:, 0])
            nc.sync.dma_start(wkT[:], wkT_d.ap())
            nc.sync.dma_start(wvT[:], wvT_d.ap())
            for tb in range(1, 4):
                nc.sync.dma_start(xT[:, tb], xT_d.ap()[:, tb])
            nc.sync.dma_start(woT[:], woT_d.ap())
            # scalar ring: only tiny constants (a big transfer here would
            # stall the ACT sequencer mid-dma_start and block the exps)
            nc.scalar.dma_start(bq[:], bq_d.ap())
            nc.scalar.dma_start(bk[:], bk_d.ap())
            nc.scalar.dma_start(mask[:], mask_d.ap())

            # ---- ACT table preload: dummy exp while DMAs stream ----
            wexp = wsp.tile([1, 512], f32, name="wexp")
            nc.scalar.activation(wexp[:], ones_bf[0:1, :], Exp)
            nc.sync.dma_start(warm_d.ap()[1:2, :], wexp[:])

            # ---- PE warmup: keep the array busy (and HAM warming) until
            # the first projection's inputs arrive (~6-7us) ----
            NWARM = 16  # cold matmuls bridging engine start (~7.7us) to
            # x-tb0 arrival (~14.4us); keeps HAM warming the whole time
            psW = mm.tile([128, 512], f32, tag="mm", name="psW")
            for i in range(NWARM):
                nc.tensor.matmul(
                    psW,
                    lhsT=ones_bf[:, 0:128],
                    rhs=ones_bf[:],
                    start=(i == 0),
                    stop=(i == NWARM - 1),
                    skip_group_check=True,
                )
            wstg = wsp.tile([1, 512], f32, name="wstg")
            nc.vector.tensor_copy(wstg[:], psW[0:1, :])
            nc.sync.dma_start(warm_d.ap()[0:1, :], wstg[:])

            # ---- filler units: (est_pe_ns, emit_fn) ----
            def qk_proj_units(wsb, bvec, dst, t, qb):
                cell = {}

                def mk(mc):
                    def fn():
                        if mc == 0:
                            cell["ps"] = mm.tile(
                                [128, 512], f32, tag="mm", name="psqk"
                            )
                        nc.tensor.matmul(
                            cell["ps"],
                            lhsT=wsb[:, mc, ts(t, 128)],
                            rhs=xT[:, qb, mc, :],
                            start=(mc == 0),
                            stop=(mc == 7),
                            skip_group_check=True,
                        )
                        if mc == 7:
                            nc.vector.tensor_scalar_add(
                                dst[:, t, ts(qb, 512)],
                                cell["ps"],
                                bvec[:, t : t + 1],
                            )

                    return (270, fn)

                return [mk(mc) for mc in range(8)]

            def v_proj_units(st):
                cell = {}

                def mk(mc):
                    def fn():
                        if mc == 0:
                            cell["ps"] = mm.tile(
                                [128, 512], f32, tag="mm", name="psv"
                            )
                        nc.tensor.matmul(
                            cell["ps"][:, 0:256],
                            lhsT=xT[:, st // 4, mc, ts(st % 4, 128)],
                            rhs=wvT[:, mc, :],
                            start=(mc == 0),
                            stop=(mc == 7),
                            skip_group_check=True,
                        )
                        if mc == 7:
                            nc.vector.tensor_copy(
                                v4[:, st, :, 0:64],
                                cell["ps"][:, 0:256].rearrange(
                                    "p (h d) -> p h d", h=4
                                ),
                            )

                    return (160, fn)

                return [mk(mc) for mc in range(8)]

            op_count = [0]

            def outproj_units(sb):
                units = []
                for jt in range(8):

                    def fn(jt=jt):
                        ps = mm.tile([128, 512], f32, tag="mm", name="psop")
                        for dchunk in range(2):
                            nc.tensor.matmul(
                                ps,
                                lhsT=woT[:, dchunk, ts(jt, 128)],
                                rhs=oT[:, dchunk, ts(sb, 512)],
                                start=(dchunk == 0),
                                stop=(dchunk == 1),
                                skip_group_check=True,
                            )
                        stg = stp.tile([128, 512], bf16, tag="st", name="stg")
                        # sb 3 and 1 drain in the end-phase where ACT is
                        # idle (exps done) and DVE is the tail bottleneck
                        if sb in (3, 1):
                            nc.scalar.copy(stg[:], ps)
                        else:
                            nc.vector.tensor_copy(stg[:], ps)
                        # alternate HWDGE rings so the final output DMAs
                        # drain two-wide instead of piling on one FIFO
                        dma_eng = nc.sync if jt % 2 == 0 else nc.scalar
                        dma_eng.dma_start(
                            out_d.ap()[ts(jt, 128), ts(sb, 512)], stg[:]
                        )

                    units.append((560, fn))
                return units

            filler = []  # list of (cost, fn), consumed front-first
            consumed = [0]

            # hold back a few units so the PE still has queued work during
            # the final block's normalization chain (released at the end)
            reserve = [5]

            def drain(budget_ns):
                spent = 0
                while len(filler) > reserve[0] and spent < budget_ns:
                    cost, fn = filler.pop(0)
                    fn()
                    consumed[0] += 1
                    spent += cost

            def drain_until(count):
                # force-consume prerequisite units: a block's scores may
                # never be emitted into the PE FIFO ahead of the filler
                # matmuls that produce its Q/K/V (in-order queue deadlock)
                while filler and consumed[0] < count:
                    cost, fn = filler.pop(0)
                    fn()
                    consumed[0] += 1

            # ---- attention chunk pipeline ----
            class Ch:
                __slots__ = (
                    "pair", "qb", "c", "w", "q0", "dc",
                    "first", "last", "ps2", "pt",
                )

            # block order (0,2,3,1): the ACT-heaviest qb3 blocks sit
            # mid-schedule where filler (qb1 projections + unlocked
            # out-proj) is plentiful; the final blocks are the small qb1
            # ones, so the PE never starves late and HAM stays at 8/8
            chunks = []
            for qb in (0, 2, 3, 1):
                for pair in range(2):
                    nch = 4 * qb + 4
                    for c in range(nch):
                        ch = Ch()
                        ch.pair, ch.qb, ch.c = pair, qb, c
                        dc = c - 4 * qb
                        ch.dc = dc
                        ch.q0 = 128 * dc if dc >= 0 else 0
                        ch.w = 512 - ch.q0
                        ch.first = c == 0
                        ch.last = c == nch - 1
                        chunks.append(ch)

            def emit_scores(ch):
                ps2 = sc2.tile([128, 2, 512], f32, tag="sc", name="ps2")
                for hh in (0, 1):
                    prow = slice(64 * hh, 64 * hh + 64)
                    nc.tensor.matmul(
                        ps2[:, hh, : ch.w],
                        lhsT=kT[prow, ch.pair, ts(ch.c, 128)],
                        rhs=qT[prow, ch.pair, ds(512 * ch.qb + ch.q0, ch.w)],
                        start=True,
                        stop=True,
                    )
                ch.ps2 = ps2

            def emit_exp(ch):
                pt = ptp.tile([128, 2, 512], bf16, tag="pt", name="pt")
                nc.scalar.activation(pt[:, :, : ch.w], ch.ps2[:, :, : ch.w], Exp)
                if ch.dc >= 0:
                    nc.gpsimd.tensor_mul(
                        pt[:, :, 0:128],
                        pt[:, :, 0:128],
                        mask[:, None, :].to_broadcast((128, 2, 128)),
                    )
                ch.pt = pt

            blk = {}

            def emit_pv(ch):
                if ch.first:
                    blk["psA"] = po.tile([128, 512], f32, tag="po", name="psA")
                    blk["psB"] = po.tile([128, 512], f32, tag="po", name="psB")
                for hh, psO in ((0, blk["psA"]), (1, blk["psB"])):
                    nc.tensor.matmul(
                        psO[0:65, ds(ch.q0, ch.w)],
                        lhsT=v4[:, ch.c, 2 * ch.pair + hh, :],
                        rhs=ch.pt[:, hh, : ch.w],
                        start=ch.first,
                        stop=ch.last,
                        skip_group_check=True,
                    )

            def emit_norm_copies():
                # PSUM->SBUF copies issued right after the block's last PV
                # so the po slots free up quickly; the PE-side broadcast
                # matmuls are deferred past the next filler batch so the PE
                # isn't parked in FIFO behind these DVE copies
                psA, psB = blk["psA"], blk["psB"]
                oA = oup.tile([128, 512], bf16, tag="ou", name="oA")
                oB = oup.tile([128, 512], bf16, tag="ou", name="oB")
                nc.vector.tensor_copy(oA[0:65, :], psA[0:65, :])
                nc.vector.tensor_copy(oB[0:65, :], psB[0:65, :])
                return oA, oB

            def emit_norm_rest(pair, qb, oA, oB, last=False):
                psR = po.tile([128, 512], f32, tag="po", name="psR")
                nc.tensor.matmul(
                    psR[0:64, :],
                    lhsT=ones_bf[64:65, 0:64],
                    rhs=oA[64:65, :],
                    start=True,
                    stop=True,
                    skip_group_check=True,
                )
                rbA = rbp.tile([128, 512], f32, tag="rb", name="rbA")
                nc.vector.reciprocal_approx_fast(rbA[0:64, :], psR[0:64, :])
                psR2 = po.tile([128, 512], f32, tag="po", name="psR2")
                nc.tensor.matmul(
                    psR2[0:64, :],
                    lhsT=ones_bf[64:65, 0:64],
                    rhs=oB[64:65, :],
                    start=True,
                    stop=True,
                    skip_group_check=True,
                )
                rbB = rbp.tile([128, 512], f32, tag="rb", name="rbB")
                nc.vector.reciprocal_approx_fast(rbB[0:64, :], psR2[0:64, :])
                # gpsimd mul is ~2x slower than DVE but off the critical
                # path mid-kernel; for the final block the chain gates the
                # last out-proj, so use DVE there
                mul_eng = nc.vector if last else nc.gpsimd
                mul_eng.tensor_mul(
                    oT[0:64, pair, ts(qb, 512)], oA[0:64, :], rbA[0:64, :]
                )
                tmpB = tbp.tile([128, 512], bf16, tag="tb", name="tmpB")
                mul_eng.tensor_mul(tmpB[0:64, :], oB[0:64, :], rbB[0:64, :])
                nc.sync.dma_start(oT[64:128, pair, ts(qb, 512)], tmpB[0:64, :])
                if pair == 1:
                    filler.extend(outproj_units(qb))

            # ---- pre-phase: minimal projections for (pair0, qb0, chunk0) ----
            for u in qk_proj_units(wqT, bq, qT, 0, 0):
                u[1]()
            for u in qk_proj_units(wkT, bk, kT, 0, 0):
                u[1]()
            for u in v_proj_units(0):
                u[1]()

            # ---- filler schedule (dependency-ordered for block order
            # 0,2,3,1). NOTE the asymmetry: Q is per-(pair, q-block), but
            # K is per-(pair, K-RANGE) — block (p, qb) reads kT columns
            # 0..(4qb+4)*128, i.e. K ranges 0..qb cumulatively — and V is
            # per-st chunk 0..4qb+3. Out-proj units are appended as their
            # q-block completes and consumed in the lulls that follow. ----
            for st in range(1, 4):  # V st1-3: per-chunk prereq of (0,0)
                filler += v_proj_units(st)
            filler += qk_proj_units(wqT, bq, qT, 1, 0)  # before (1,0)
            filler += qk_proj_units(wkT, bk, kT, 1, 0)
            # before (0,2): Q t0 qb2, K t0 ranges 1-2, V st4-11
            filler += qk_proj_units(wqT, bq, qT, 0, 2)
            filler += qk_proj_units(wkT, bk, kT, 0, 1)
            filler += qk_proj_units(wkT, bk, kT, 0, 2)
            for st in range(4, 12):
                filler += v_proj_units(st)
            # before (1,2): Q t1 qb2, K t1 ranges 1-2
            filler += qk_proj_units(wqT, bq, qT, 1, 2)
            filler += qk_proj_units(wkT, bk, kT, 1, 1)
            filler += qk_proj_units(wkT, bk, kT, 1, 2)
            # before (0,3): Q t0 qb3, K t0 range 3, V st12-15
            filler += qk_proj_units(wqT, bq, qT, 0, 3)
            filler += qk_proj_units(wkT, bk, kT, 0, 3)
            for st in range(12, 16):
                filler += v_proj_units(st)
            # before (1,3): Q t1 qb3, K t1 range 3
            filler += qk_proj_units(wqT, bq, qT, 1, 3)
            filler += qk_proj_units(wkT, bk, kT, 1, 3)
            # before (0,1)/(1,1): just Q (K ranges 0-1 already done)
            filler += qk_proj_units(wqT, bq, qT, 0, 1)
            filler += qk_proj_units(wqT, bq, qT, 1, 1)

            # units that must be consumed before each block's first scores
            # (cumulative position in the dependency-ordered filler list)
            prereq = {
                (1, 0): 40,
                (0, 2): 128,
                (1, 2): 152,
                (0, 3): 200,
                (1, 3): 216,
                (0, 1): 224,
                (1, 1): 232,
            }
            # V st1-3 sit at filler positions 0..23; (p0,qb0) chunk c's PV
            # needs V st c emitted first (PE FIFO would deadlock otherwise)
            pv_prereq = {(0, 0, 1): 8, (0, 0, 2): 16, (0, 0, 3): 24}

            emit_scores(chunks[0])
            emit_exp(chunks[0])
            nchunks_total = len(chunks)
            debt = [0.0]
            pending = []
            for i, ch in enumerate(chunks):
                if i + 1 < len(chunks):
                    nxt = chunks[i + 1]
                    if nxt.first and (nxt.pair, nxt.qb) in prereq:
                        drain_until(prereq[(nxt.pair, nxt.qb)])
                    emit_scores(nxt)
                    emit_exp(nxt)
                key = (ch.pair, ch.qb, ch.c)
                if key in pv_prereq:
                    drain_until(pv_prereq[key])
                # keep PE fed while ACT computes exp(ch): spread the
                # remaining filler evenly over the remaining chunks so the
                # late ACT-heavy blocks never starve the PE. Accumulate the
                # budget and release it in >=1.2us batches: consecutive
                # same-shape matmuls keep LDWEIGHTS prefetch working (a
                # lone filler MM after a PV pays its weight load exposed).
                remaining = sum(c for c, _ in filler)
                left = nchunks_total - i
                debt[0] += max(300 + 0.45 * ch.w, 1.05 * remaining / left)
                if debt[0] >= 1200 or left <= 2:
                    drain(debt[0])
                    debt[0] = 0.0
                while pending:
                    emit_norm_rest(*pending.pop(0))
                emit_pv(ch)
                if ch.last:
                    oA, oB = emit_norm_copies()
                    pending.append(
                        (ch.pair, ch.qb, oA, oB, i == nchunks_total - 1)
                    )

            # tail: a bit of reserved PE work covers the final norm chain,
            # then the rest of the out-proj (at least the last q-block's)
            reserve[0] = 0
            drain(2500)
            while pending:
                emit_norm_rest(*pending.pop(0))
            while filler:
                filler.pop(0)[1]()

            if DEBUG:
                nc.sync.dma_start(qT_o.ap(), qT[:])
                nc.sync.dma_start(kT_o.ap(), kT[:])
                nc.sync.dma_start(v4_o.ap(), v4[:])
                nc.sync.dma_start(oT_o.ap(), oT[:])

    nc.compile()
    return nc


def _get_bass():
    if "nc" not in _cache:
        _cache["nc"] = _build_bass()
    return _cache["nc"]


def _shard_inputs(x, Wq, bq, Wk, bk, Wv, bv, Wo, bo):
    import ml_dtypes

    bft = ml_dtypes.bfloat16
    x = np.asarray(x, dtype=np.float32)
    Wq = np.asarray(Wq, dtype=np.float32)
    Wk = np.asarray(Wk, dtype=np.float32)
    Wv = np.asarray(Wv, dtype=np.float32)
    Wo = np.asarray(Wo, dtype=np.float32)
    bq = np.asarray(bq, dtype=np.float32)
    bk = np.asarray(bk, dtype=np.float32)

    kk = np.arange(128)[:, None]
    qq = np.arange(128)[None, :]
    mask128 = (kk <= qq).astype(bft)

    def dev_layout(mat, npart_groups):
        # [G*128, F] -> [128, G, F] (partition-major, contiguous/partition)
        g, f = npart_groups, mat.shape[1]
        return np.ascontiguousarray(
            mat.reshape(g, 128, f).transpose(1, 0, 2)
        )

    def x_layout(xTb):
        # [1024, 2048] -> [128, 4 tb, 8 o, 512]
        t = xTb.reshape(8, 128, 4, 512).transpose(1, 2, 0, 3)
        return np.ascontiguousarray(t)

    xT = [x_layout(x[b].T.astype(bft)) for b in range(x.shape[0])]
    in_maps = []
    for c in range(NCORES):
        b, g = divmod(c, 4)
        sl = slice(DL * g, DL * (g + 1))
        in_maps.append(
            {
                "xT": xT[b],
                "wqT": dev_layout((Wq[sl].T * 0.125).astype(bft), 8),
                "wkT": dev_layout(Wk[sl].T.astype(bft), 8),
                "wvT": dev_layout(Wv[sl].T.astype(bft), 8),
                "woT": dev_layout(Wo[:, sl].T.astype(bft), 2),
                "bq": np.ascontiguousarray(
                    (bq[sl] * 0.125).reshape(2, 128).T
                ),
                "bk": np.ascontiguousarray(bk[sl].reshape(2, 128).T),
                "mask": mask128,
            }
        )
    return in_maps


def kernel(x, Wq, bq, Wk, bk, Wv, bv, Wo, bo):
    global LAST_EXEC_TIME_NS, LAST_TRACE_PATH
    from concourse.bass_utils import run_bass_kernel_spmd

    nc = _get_bass()
    in_maps = _shard_inputs(x, Wq, bq, Wk, bk, Wv, bv, Wo, bo)

    trace = os.environ.get("KERNEL_TRACE", "0") == "1"
    res = run_bass_kernel_spmd(
        nc, in_maps, core_ids=list(range(NCORES)), trace=trace
    )
    LAST_EXEC_TIME_NS = res.exec_time_ns
    if res.instructions_and_trace is not None:
        LAST_TRACE_PATH = res.instructions_and_trace[1]

    bo_full = (
        np.asarray(bo, np.float64)
        + np.asarray(bv, np.float64) @ np.asarray(Wo, np.float64).T
    ).astype(np.float32)

    B = 2
    out = np.empty((B, S, D), dtype=np.float32)
    for b in range(B):
        acc = res.results[4 * b]["outT"].astype(np.float32)
        for g in range(1, 4):
            acc = acc + res.results[4 * b + g]["outT"].astype(np.float32)
        out[b] = acc.T + bo_full[None, :]
    return out


# revision 46
# speedup vs baseline: 1.0438x; 1.0182x over previous
# Causal self-attention on 8 TRN2 NeuronCores.
#
# Sharding (data + tensor parallel per the hint):
#   core c -> batch b = c // 4, head group g = c % 4 (4 heads of 64 dims = 256).
#   Wq/Wk/Wv split column-wise per head group; Wo row-wise. Each core emits a
#   partial [D, S] output in bf16; the host sums the 4 partials per batch
#   (the "all-reduce" of row-parallel sharding), transposes, and adds
#   bo' = bo + bv @ Wo.T (the V-bias commutes through softmax-normalize +
#   out-projection, so it is folded into the host-side bias).
#
# Device kernel (per core), all matmuls bf16 (PE streams 1 col/cycle for
# every dtype, so bf16 costs the same PE time as fp32 but halves DMA/SBUF
# and unlocks 2x DVE modes; tolerance is 2e-2, bf16 lands ~1e-3):
#   xT [D, S] resident in SBUF (bf16).
#   QT/KT [128 = 2 heads x 64, pair, S] = W x; bias added on DVE during the
#     PSUM->SBUF stage via per-partition tensor_scalar_add (no bias matmuls;
#     1/8 softmax scale folded into Wq/bq on the host).
#   V [S, 4 heads, 64+1] with a ones column (rowsum rides along in PV).
#   Attention is one global chunk pipeline across all (pair, q-block)
#   blocks: per chunk, a row-tiled pair of K=64 score matmuls (two heads
#   concurrently in the PE array), exp on ACT (PSUM->SBUF bf16), causal
#   mask multiply on GpSimd (diag chunks), then PV accumulation.
#   The PE stream is emitted with scores lookahead 1 (scores of chunk i+1
#   precede PV of chunk i) and a calibrated amount of "filler" matmuls
#   (projections for later blocks + out-projection of finished q-blocks)
#   between them, so the PE never idles waiting for ACT and the HAM clock
#   gate stays at 2.4 GHz. ACT's exp stream is the secondary resource
#   (~68us vs ~100us PE); fillers are deferred so the late, ACT-heavy
#   blocks still have PE work available.
#   Normalization: rowsums -> PE broadcast (K=1 matmul) -> DVE approx
#   reciprocal -> multiply (head 0 on DVE in place, head 1 via tmp + DMA to
#   partitions 64..127).
#   Out projection: 2 accumulating matmuls per [128,512] tile, staged
#   PSUM->SBUF bf16 alternating DVE/ACT, DMA'd out bf16 (no bias on device).

import os

import numpy as np

S = 2048
D = 1024
DL = 256  # local head dims (4 heads x 64)
NCORES = 8

_cache = {}
LAST_EXEC_TIME_NS = None
LAST_TRACE_PATH = None


DEBUG = os.environ.get("KERNEL_DEBUG", "0") == "1"


def _build_bass():
    from concourse import bacc
    import concourse.tile as tile
    import concourse.mybir as mybir
    from concourse.bass import ts, ds

    f32 = mybir.dt.float32
    bf16 = mybir.dt.bfloat16
    Exp = mybir.ActivationFunctionType.Exp

    nc = bacc.Bacc("TRN2", target_bir_lowering=False, debug=False)

    # DRAM layouts mirror the SBUF tiles (partition-major, contiguous per
    # partition) so each input DMA lowers to 128 large descriptors instead
    # of thousands of 512B row fragments
    xT_d = nc.dram_tensor("xT", [128, 4, 8, 512], bf16, kind="ExternalInput")
    wqT_d = nc.dram_tensor("wqT", [128, 8, DL], bf16, kind="ExternalInput")
    wkT_d = nc.dram_tensor("wkT", [128, 8, DL], bf16, kind="ExternalInput")
    wvT_d = nc.dram_tensor("wvT", [128, 8, DL], bf16, kind="ExternalInput")
    woT_d = nc.dram_tensor("woT", [128, 2, D], bf16, kind="ExternalInput")
    bq_d = nc.dram_tensor("bq", [128, 2], f32, kind="ExternalInput")
    bk_d = nc.dram_tensor("bk", [128, 2], f32, kind="ExternalInput")
    mask_d = nc.dram_tensor("mask", [128, 128], bf16, kind="ExternalInput")
    out_d = nc.dram_tensor("outT", [D, S], bf16, kind="ExternalOutput")
    warm_d = nc.dram_tensor("warm", [2, 512], f32, kind="ExternalOutput")
    if DEBUG:
        qT_o = nc.dram_tensor("qT_o", [128, 2, S], bf16, kind="ExternalOutput")
        kT_o = nc.dram_tensor("kT_o", [128, 2, S], bf16, kind="ExternalOutput")
        v4_o = nc.dram_tensor("v4_o", [128, 16, 4, 65], bf16, kind="ExternalOutput")
        oT_o = nc.dram_tensor("oT_o", [128, 2, S], bf16, kind="ExternalOutput")

    with tile.TileContext(nc) as tc:
        with (
            tc.tile_pool(name="persist", bufs=1) as persist,
            tc.tile_pool(name="ptp", bufs=4) as ptp,
            tc.tile_pool(name="oup", bufs=2) as oup,
            tc.tile_pool(name="rbp", bufs=2) as rbp,
            tc.tile_pool(name="stp", bufs=3) as stp,
            tc.tile_pool(name="tbp", bufs=2) as tbp,
            tc.tile_pool(name="wsp", bufs=1) as wsp,
            tc.tile_pool(name="sc2", bufs=2, space="PSUM") as sc2,
            tc.tile_pool(name="mm", bufs=2, space="PSUM") as mm,
            tc.tile_pool(name="po", bufs=2, space="PSUM") as po,
        ):
            # ---- persistent SBUF tensors ----
            # xT is tb-major [p, tb, o, f2] so each quarter's DMA is one
            # contiguous 8KB descriptor per partition on both sides
            xT = persist.tile([128, 4, 8, 512], bf16, name="xT_sb")
            wqT = persist.tile([128, 8, DL], bf16, name="wqT_sb")
            wkT = persist.tile([128, 8, DL], bf16, name="wkT_sb")
            wvT = persist.tile([128, 8, DL], bf16, name="wvT_sb")
            woT = persist.tile([128, 2, D], bf16, name="woT_sb")
            bq = persist.tile([128, 2], f32, name="bq_sb")
            bk = persist.tile([128, 2], f32, name="bk_sb")
            mask = persist.tile([128, 128], bf16, name="mask_sb")
            ones_bf = persist.tile([128, 512], bf16, name="ones_bf")
            qT = persist.tile([128, 2, S], bf16, name="qT_sb")
            kT = persist.tile([128, 2, S], bf16, name="kT_sb")
            v4 = persist.tile([128, 16, 4, 65], bf16, name="v4_sb")
            oT = persist.tile([128, 2, S], bf16, name="oT_sb")

            # ---- input DMAs ----
            # gpsimd starts earliest and is otherwise idle: constants there
            nc.gpsimd.memset(ones_bf[:], 1.0)
            nc.gpsimd.memset(v4[:, :, :, 64:65], 1.0)
            # sync ring, in first-use order: wq+x(qb0) gate the pre-phase,
            # wk/wv before the rest of x, wo (out-proj) last
            nc.sync.dma_start(wqT[:], wqT_d.ap())
            nc.sync.dma_start(xT[:, 0], xT_d.ap()[:, 0])
            nc.sync.dma_start(wkT[:], wkT_d.ap())
            nc.sync.dma_start(wvT[:], wvT_d.ap())
            for tb in range(1, 4):
                nc.sync.dma_start(xT[:, tb], xT_d.ap()[:, tb])
            nc.sync.dma_start(woT[:], woT_d.ap())
            # scalar ring: only tiny constants (a big transfer here would
            # stall the ACT sequencer mid-dma_start and block the exps)
            nc.scalar.dma_start(bq[:], bq_d.ap())
            nc.scalar.dma_start(bk[:], bk_d.ap())
            nc.scalar.dma_start(mask[:], mask_d.ap())

            # ---- ACT table preload: dummy exp while DMAs stream ----
            wexp = wsp.tile([1, 512], f32, name="wexp")
            nc.scalar.activation(wexp[:], ones_bf[0:1, :], Exp)
            nc.sync.dma_start(warm_d.ap()[1:2, :], wexp[:])

            # ---- PE warmup: keep the array busy (and HAM warming) until
            # the first projection's inputs arrive (~6-7us) ----
            NWARM = 19  # cold matmuls bridging engine start (~7.7us) to
            # x-tb0 arrival (~14.4us); keeps HAM warming the whole time
            psW = mm.tile([128, 512], f32, tag="mm", name="psW")
            for i in range(NWARM):
                nc.tensor.matmul(
                    psW,
                    lhsT=ones_bf[:, 0:128],
                    rhs=ones_bf[:],
                    start=(i == 0),
                    stop=(i == NWARM - 1),
                    skip_group_check=True,
                )
            wstg = wsp.tile([1, 512], f32, name="wstg")
            nc.vector.tensor_copy(wstg[:], psW[0:1, :])
            nc.sync.dma_start(warm_d.ap()[0:1, :], wstg[:])

            # ---- filler units: (est_pe_ns, emit_fn) ----
            def qk_proj_units(wsb, bvec, dst, t, qb):
                cell = {}

                def mk(mc):
                    def fn():
                        if mc == 0:
                            cell["ps"] = mm.tile(
                                [128, 512], f32, tag="mm", name="psqk"
                            )
                        nc.tensor.matmul(
                            cell["ps"],
                            lhsT=wsb[:, mc, ts(t, 128)],
                            rhs=xT[:, qb, mc, :],
                            start=(mc == 0),
                            stop=(mc == 7),
                            skip_group_check=True,
                        )
                        if mc == 7:
                            nc.vector.tensor_scalar_add(
                                dst[:, t, ts(qb, 512)],
                                cell["ps"],
                                bvec[:, t : t + 1],
                            )

                    return (270, fn)

                return [mk(mc) for mc in range(8)]

            def v_proj_units(st):
                cell = {}

                def mk(mc):
                    def fn():
                        if mc == 0:
                            cell["ps"] = mm.tile(
                                [128, 512], f32, tag="mm", name="psv"
                            )
                        nc.tensor.matmul(
                            cell["ps"][:, 0:256],
                            lhsT=xT[:, st // 4, mc, ts(st % 4, 128)],
                            rhs=wvT[:, mc, :],
                            start=(mc == 0),
                            stop=(mc == 7),
                            skip_group_check=True,
                        )
                        if mc == 7:
                            nc.vector.tensor_copy(
                                v4[:, st, :, 0:64],
                                cell["ps"][:, 0:256].rearrange(
                                    "p (h d) -> p h d", h=4
                                ),
                            )

                    return (160, fn)

                return [mk(mc) for mc in range(8)]

            op_count = [0]

            def outproj_units(sb):
                units = []
                for jt in range(8):

                    def fn(jt=jt):
                        ps = mm.tile([128, 512], f32, tag="mm", name="psop")
                        for dchunk in range(2):
                            nc.tensor.matmul(
                                ps,
                                lhsT=woT[:, dchunk, ts(jt, 128)],
                                rhs=oT[:, dchunk, ts(sb, 512)],
                                start=(dchunk == 0),
                                stop=(dchunk == 1),
                                skip_group_check=True,
                            )
                        stg = stp.tile([128, 512], bf16, tag="st", name="stg")
                        # sb 3 and 1 drain in the end-phase where ACT is
                        # idle (exps done) and DVE is the tail bottleneck
                        if sb in (3, 1):
                            nc.scalar.copy(stg[:], ps)
                        else:
                            nc.vector.tensor_copy(stg[:], ps)
                        # alternate HWDGE rings so the final output DMAs
                        # drain two-wide instead of piling on one FIFO
                        dma_eng = nc.sync if jt % 2 == 0 else nc.scalar
                        dma_eng.dma_start(
                            out_d.ap()[ts(jt, 128), ts(sb, 512)], stg[:]
                        )

                    units.append((560, fn))
                return units

            filler = []  # list of (cost, fn), consumed front-first
            consumed = [0]

            # hold back a few units so the PE still has queued work during
            # the final block's normalization chain (released at the end)
            reserve = [5]

            def drain(budget_ns):
                spent = 0
                while len(filler) > reserve[0] and spent < budget_ns:
                    cost, fn = filler.pop(0)
                    fn()
                    consumed[0] += 1
                    spent += cost

            def drain_until(count):
                # force-consume prerequisite units: a block's scores may
                # never be emitted into the PE FIFO ahead of the filler
                # matmuls that produce its Q/K/V (in-order queue deadlock)
                while filler and consumed[0] < count:
                    cost, fn = filler.pop(0)
                    fn()
                    consumed[0] += 1

            # ---- attention chunk pipeline ----
            class Ch:
                __slots__ = (
                    "pair", "qb", "c", "w", "q0", "dc",
                    "first", "last", "ps2", "pt",
                )

            # block order (0,2,3,1): the ACT-heaviest qb3 blocks sit
            # mid-schedule where filler (qb1 projections + unlocked
            # out-proj) is plentiful; the final blocks are the small qb1
            # ones, so the PE never starves late and HAM stays at 8/8
            chunks = []
            for qb in (0, 2, 3, 1):
                for pair in range(2):
                    nch = 4 * qb + 4
                    for c in range(nch):
                        ch = Ch()
                        ch.pair, ch.qb, ch.c = pair, qb, c
                        dc = c - 4 * qb
                        ch.dc = dc
                        ch.q0 = 128 * dc if dc >= 0 else 0
                        ch.w = 512 - ch.q0
                        ch.first = c == 0
                        ch.last = c == nch - 1
                        chunks.append(ch)

            def emit_scores(ch):
                ps2 = sc2.tile([128, 2, 512], f32, tag="sc", name="ps2")
                for hh in (0, 1):
                    prow = slice(64 * hh, 64 * hh + 64)
                    nc.tensor.matmul(
                        ps2[:, hh, : ch.w],
                        lhsT=kT[prow, ch.pair, ts(ch.c, 128)],
                        rhs=qT[prow, ch.pair, ds(512 * ch.qb + ch.q0, ch.w)],
                        start=True,
                        stop=True,
                    )
                ch.ps2 = ps2

            def emit_exp(ch):
                pt = ptp.tile([128, 2, 512], bf16, tag="pt", name="pt")
                nc.scalar.activation(pt[:, :, : ch.w], ch.ps2[:, :, : ch.w], Exp)
                if ch.dc >= 0:
                    nc.gpsimd.tensor_mul(
                        pt[:, :, 0:128],
                        pt[:, :, 0:128],
                        mask[:, None, :].to_broadcast((128, 2, 128)),
                    )
                ch.pt = pt

            blk = {}

            def emit_pv(ch):
                if ch.first:
                    blk["psA"] = po.tile([128, 512], f32, tag="po", name="psA")
                    blk["psB"] = po.tile([128, 512], f32, tag="po", name="psB")
                for hh, psO in ((0, blk["psA"]), (1, blk["psB"])):
                    nc.tensor.matmul(
                        psO[0:65, ds(ch.q0, ch.w)],
                        lhsT=v4[:, ch.c, 2 * ch.pair + hh, :],
                        rhs=ch.pt[:, hh, : ch.w],
                        start=ch.first,
                        stop=ch.last,
                        skip_group_check=True,
                    )

            def emit_norm_copies():
                # PSUM->SBUF copies issued right after the block's last PV
                # so the po slots free up quickly; the PE-side broadcast
                # matmuls are deferred past the next filler batch so the PE
                # isn't parked in FIFO behind these DVE copies
                psA, psB = blk["psA"], blk["psB"]
                oA = oup.tile([128, 512], bf16, tag="ou", name="oA")
                oB = oup.tile([128, 512], bf16, tag="ou", name="oB")
                nc.vector.tensor_copy(oA[0:65, :], psA[0:65, :])
                nc.vector.tensor_copy(oB[0:65, :], psB[0:65, :])
                return oA, oB

            def emit_norm_rest(pair, qb, oA, oB, last=False):
                psR = po.tile([128, 512], f32, tag="po", name="psR")
                nc.tensor.matmul(
                    psR[0:64, :],
                    lhsT=ones_bf[64:65, 0:64],
                    rhs=oA[64:65, :],
                    start=True,
                    stop=True,
                    skip_group_check=True,
                )
                rbA = rbp.tile([128, 512], f32, tag="rb", name="rbA")
                nc.vector.reciprocal_approx_fast(rbA[0:64, :], psR[0:64, :])
                psR2 = po.tile([128, 512], f32, tag="po", name="psR2")
                nc.tensor.matmul(
                    psR2[0:64, :],
                    lhsT=ones_bf[64:65, 0:64],
                    rhs=oB[64:65, :],
                    start=True,
                    stop=True,
                    skip_group_check=True,
                )
                rbB = rbp.tile([128, 512], f32, tag="rb", name="rbB")
                nc.vector.reciprocal_approx_fast(rbB[0:64, :], psR2[0:64, :])
                # gpsimd mul is ~2x slower than DVE but off the critical
                # path mid-kernel; for the final block the chain gates the
                # last out-proj, so use DVE there
                mul_eng = nc.vector if last else nc.gpsimd
                mul_eng.tensor_mul(
                    oT[0:64, pair, ts(qb, 512)], oA[0:64, :], rbA[0:64, :]
                )
                tmpB = tbp.tile([128, 512], bf16, tag="tb", name="tmpB")
                mul_eng.tensor_mul(tmpB[0:64, :], oB[0:64, :], rbB[0:64, :])
                nc.sync.dma_start(oT[64:128, pair, ts(qb, 512)], tmpB[0:64, :])
                if pair == 1:
                    filler.extend(outproj_units(qb))

            # ---- pre-phase: minimal projections for (pair0, qb0, chunk0) ----
            for u in qk_proj_units(wqT, bq, qT, 0, 0):
                u[1]()
            for u in qk_proj_units(wkT, bk, kT, 0, 0):
                u[1]()
            for u in v_proj_units(0):
                u[1]()

            # ---- filler schedule (dependency-ordered for block order
            # 0,2,3,1). NOTE the asymmetry: Q is per-(pair, q-block), but
            # K is per-(pair, K-RANGE) — block (p, qb) reads kT columns
            # 0..(4qb+4)*128, i.e. K ranges 0..qb cumulatively — and V is
            # per-st chunk 0..4qb+3. Out-proj units are appended as their
            # q-block completes and consumed in the lulls that follow. ----
            for st in range(1, 4):  # V st1-3: per-chunk prereq of (0,0)
                filler += v_proj_units(st)
            filler += qk_proj_units(wqT, bq, qT, 1, 0)  # before (1,0)
            filler += qk_proj_units(wkT, bk, kT, 1, 0)
            # before (0,2): Q t0 qb2, K t0 ranges 1-2, V st4-11
            filler += qk_proj_units(wqT, bq, qT, 0, 2)
            filler += qk_proj_units(wkT, bk, kT, 0, 1)
            filler += qk_proj_units(wkT, bk, kT, 0, 2)
            for st in range(4, 12):
                filler += v_proj_units(st)
            # before (1,2): Q t1 qb2, K t1 ranges 1-2
            filler += qk_proj_units(wqT, bq, qT, 1, 2)
            filler += qk_proj_units(wkT, bk, kT, 1, 1)
            filler += qk_proj_units(wkT, bk, kT, 1, 2)
            # before (0,3): Q t0 qb3, K t0 range 3, V st12-15
            filler += qk_proj_units(wqT, bq, qT, 0, 3)
            filler += qk_proj_units(wkT, bk, kT, 0, 3)
            for st in range(12, 16):
                filler += v_proj_units(st)
            # before (1,3): Q t1 qb3, K t1 range 3
            filler += qk_proj_units(wqT, bq, qT, 1, 3)
            filler += qk_proj_units(wkT, bk, kT, 1, 3)
            # before (0,1)/(1,1): just Q (K ranges 0-1 already done)
            filler += qk_proj_units(wqT, bq, qT, 0, 1)
            filler += qk_proj_units(wqT, bq, qT, 1, 1)

            # units that must be consumed before each block's first scores
            # (cumulative position in the dependency-ordered filler list)
            prereq = {
                (1, 0): 40,
                (0, 2): 128,
                (1, 2): 152,
                (0, 3): 200,
                (1, 3): 216,
                (0, 1): 224,
                (1, 1): 232,
            }
            # V st1-3 sit at filler positions 0..23; (p0,qb0) chunk c's PV
            # needs V st c emitted first (PE FIFO would deadlock otherwise)
            pv_prereq = {(0, 0, 1): 8, (0, 0, 2): 16, (0, 0, 3): 24}

            emit_scores(chunks[0])
            emit_exp(chunks[0])
            nchunks_total = len(chunks)
            debt = [0.0]
            pending = []
            for i, ch in enumerate(chunks):
                if i + 1 < len(chunks):
                    nxt = chunks[i + 1]
                    if nxt.first and (nxt.pair, nxt.qb) in prereq:
                        drain_until(prereq[(nxt.pair, nxt.qb)])
                    emit_scores(nxt)
                    emit_exp(nxt)
                key = (ch.pair, ch.qb, ch.c)
                if key in pv_prereq:
                    drain_until(pv_prereq[key])
                # keep PE fed while ACT computes exp(ch): spread the
                # remaining filler evenly over the remaining chunks so the
                # late ACT-heavy blocks never starve the PE. Accumulate the
                # budget and release it in >=1.2us batches: consecutive
                # same-shape matmuls keep LDWEIGHTS prefetch working (a
                # lone filler MM after a PV pays its weight load exposed).
                remaining = sum(c for c, _ in filler)
                left = nchunks_total - i
                debt[0] += max(300 + 0.45 * ch.w, 1.05 * remaining / left)
                if debt[0] >= 1200 or left <= 2:
                    drain(debt[0])
                    debt[0] = 0.0
                while pending:
                    emit_norm_rest(*pending.pop(0))
                emit_pv(ch)
                if ch.last:
                    oA, oB = emit_norm_copies()
                    pending.append(
                        (ch.pair, ch.qb, oA, oB, i == nchunks_total - 1)
                    )

            # tail: a bit of reserved PE work covers the final norm chain,
            # then the rest of the out-proj (at least the last q-block's)
            reserve[0] = 0
            drain(2500)
            while pending:
                emit_norm_rest(*pending.pop(0))
            while filler:
                filler.pop(0)[1]()

            if DEBUG:
                nc.sync.dma_start(qT_o.ap(), qT[:])
                nc.sync.dma_start(kT_o.ap(), kT[:])
                nc.sync.dma_start(v4_o.ap(), v4[:])
                nc.sync.dma_start(oT_o.ap(), oT[:])

    nc.compile()
    return nc


def _get_bass():
    if "nc" not in _cache:
        _cache["nc"] = _build_bass()
    return _cache["nc"]


def _shard_inputs(x, Wq, bq, Wk, bk, Wv, bv, Wo, bo):
    import ml_dtypes

    bft = ml_dtypes.bfloat16
    x = np.asarray(x, dtype=np.float32)
    Wq = np.asarray(Wq, dtype=np.float32)
    Wk = np.asarray(Wk, dtype=np.float32)
    Wv = np.asarray(Wv, dtype=np.float32)
    Wo = np.asarray(Wo, dtype=np.float32)
    bq = np.asarray(bq, dtype=np.float32)
    bk = np.asarray(bk, dtype=np.float32)

    kk = np.arange(128)[:, None]
    qq = np.arange(128)[None, :]
    mask128 = (kk <= qq).astype(bft)

    def dev_layout(mat, npart_groups):
        # [G*128, F] -> [128, G, F] (partition-major, contiguous/partition)
        g, f = npart_groups, mat.shape[1]
        return np.ascontiguousarray(
            mat.reshape(g, 128, f).transpose(1, 0, 2)
        )

    def x_layout(xTb):
        # [1024, 2048] -> [128, 4 tb, 8 o, 512]
        t = xTb.reshape(8, 128, 4, 512).transpose(1, 2, 0, 3)
        return np.ascontiguousarray(t)

    xT = [x_layout(x[b].T.astype(bft)) for b in range(x.shape[0])]
    in_maps = []
    for c in range(NCORES):
        b, g = divmod(c, 4)
        sl = slice(DL * g, DL * (g + 1))
        in_maps.append(
            {
                "xT": xT[b],
                "wqT": dev_layout((Wq[sl].T * 0.125).astype(bft), 8),
                "wkT": dev_layout(Wk[sl].T.astype(bft), 8),
                "wvT": dev_layout(Wv[sl].T.astype(bft), 8),
                "woT": dev_layout(Wo[:, sl].T.astype(bft), 2),
                "bq": np.ascontiguousarray(
                    (bq[sl] * 0.125).reshape(2, 128).T
                ),
                "bk": np.ascontiguousarray(bk[sl].reshape(2, 128).T),
                "mask": mask128,
            }
        )
    return in_maps


def kernel(x, Wq, bq, Wk, bk, Wv, bv, Wo, bo):
    global LAST_EXEC_TIME_NS, LAST_TRACE_PATH
    from concourse.bass_utils import run_bass_kernel_spmd

    nc = _get_bass()
    in_maps = _shard_inputs(x, Wq, bq, Wk, bk, Wv, bv, Wo, bo)

    trace = os.environ.get("KERNEL_TRACE", "0") == "1"
    res = run_bass_kernel_spmd(
        nc, in_maps, core_ids=list(range(NCORES)), trace=trace
    )
    LAST_EXEC_TIME_NS = res.exec_time_ns
    if res.instructions_and_trace is not None:
        LAST_TRACE_PATH = res.instructions_and_trace[1]

    bo_full = (
        np.asarray(bo, np.float64)
        + np.asarray(bv, np.float64) @ np.asarray(Wo, np.float64).T
    ).astype(np.float32)

    B = 2
    out = np.empty((B, S, D), dtype=np.float32)
    for b in range(B):
        acc = res.results[4 * b]["outT"].astype(np.float32)
        for g in range(1, 4):
            acc = acc + res.results[4 * b + g]["outT"].astype(np.float32)
        out[b] = acc.T + bo_full[None, :]
    return out
